# revision 13
# baseline (speedup 1.0000x reference)
"""Trainium2 Bass kernel for a 3-layer MLP classifier.

  x:[16,512,256,5,5] -> rows [8192, 6400]
  out = relu(relu(x@W1+b1)@W2+b2)@W3+b3 -> [16, 512, 21]

Data-parallel over 8 NeuronCores: 1024 rows/core, weights replicated.

Per-core pipeline, bf16 compute (HW-measured: bf16 matmul N=512 = 99ns,
bf16 128x128 PE transpose = 86ns, DVE [128,1024]-bf16 PSUM evac ~0.8us):
  - x rows DMA'd naturally as [128 rows, 3200] f32 chunks (line rate,
    ~362 GB/s/core) and converted once to bf16 (split across DVE and ACT).
  - PE transposes 128x128 bf16 tiles of x into a bf16 PSUM bank holding two
    k-chunks ([128, 2, 512]); DVE evacuates each bank in one copy.
  - L1: psum_h1T[oi] += W1_bf16_lhsT @ xT_bf16 -> h1^T [256 ch, 512 rows];
    channel on partitions so relu+b1 is a per-partition ScalarE activation
    emitting bf16.
  - L2: lhsT=W2 chunk, rhs=h1^T -> h2^T [64, 512]; relu+b2 likewise.
  - L3: lhsT = h2^T padded to K=96 (row 64 = ones so W3ext row 64 = b3 adds
    the bias; rows 65:96 zeros), rhs = W3ext [96, 32] -> natural-orientation
    out [128 rows, 32] in PSUM f32; DVE copies cols 0:21 to SBUF; DMA out.

Weights are staged as f32 DMA loads then converted once on DVE (bf16).
"""

from contextlib import ExitStack

import numpy as np

import concourse.bass as bass
import concourse.mybir as mybir
import concourse.tile as tile
from concourse import bacc
from concourse.bass_utils import run_bass_kernel_spmd

F32 = mybir.dt.float32
BF16 = mybir.dt.bfloat16
RELU = mybir.ActivationFunctionType.Relu
IDENT = mybir.ActivationFunctionType.Identity

N_CORES = 8
ROWS_TOTAL = 16 * 512            # 8192
ROWS = ROWS_TOTAL // N_CORES     # 1024 rows per core
D_IN = 6400                      # 256 * 5 * 5
H1 = 256
H2 = 64
N_CLS = 21
N_PAD = 32                       # L3 moving dim padded (mult of 32)
K3 = 96                          # L3 contraction padded (64 + ones + zeros)

BLK = 512                        # rows per compute block (PSUM bank = 512 f32)
RSUB = BLK // 128                # 4 row sub-tiles per block
N_BLK = ROWS // BLK              # 2 blocks per core
KI = D_IN // 128                 # 50 contraction chunks
DC = 2                           # x column-chunks per row sub-tile
DCW = D_IN // DC                 # 3200 elements per chunk (1.64MB DMA per tile)
KI_PER_DC = DCW // 128           # 25


def _make_identity_bf16(nc, ident):
    nc.gpsimd.memset(ident[:], 0.0)
    nc.gpsimd.affine_select(
        out=ident,
        in_=ident,
        compare_op=mybir.AluOpType.not_equal,
        fill=1.0,
        base=0,
        pattern=[[-1, 128]],
        channel_multiplier=1,
    )


def build_program(repeat: int = 1):
    nc = bacc.Bacc("TRN2", target_bir_lowering=False, debug=False)

    x_d = nc.dram_tensor("x", [ROWS, D_IN], F32, kind="ExternalInput").ap()
    w1_d = nc.dram_tensor("W1", [D_IN, H1], F32, kind="ExternalInput").ap()
    b1_d = nc.dram_tensor("b1", [H1], F32, kind="ExternalInput").ap()
    w2_d = nc.dram_tensor("W2", [H1, H2], F32, kind="ExternalInput").ap()
    b2_d = nc.dram_tensor("b2", [H2], F32, kind="ExternalInput").ap()
    w3_d = nc.dram_tensor("W3", [H2, N_CLS], F32, kind="ExternalInput").ap()
    b3_d = nc.dram_tensor("b3", [N_CLS], F32, kind="ExternalInput").ap()
    out_d = nc.dram_tensor("out", [ROWS, N_CLS], F32, kind="ExternalOutput").ap()

    with tile.TileContext(nc) as tc, ExitStack() as ctx:
        const = ctx.enter_context(tc.tile_pool(name="const", bufs=1))
        xnat_p = ctx.enter_context(tc.tile_pool(name="xnat", bufs=5))
        xbf_p = ctx.enter_context(tc.tile_pool(name="xbf", bufs=10))
        xt_p = ctx.enter_context(tc.tile_pool(name="xt", bufs=4))
        h_p = ctx.enter_context(tc.tile_pool(name="h", bufs=4))
        o_p = ctx.enter_context(tc.tile_pool(name="o", bufs=2))
        ptp_p = ctx.enter_context(tc.tile_pool(name="ptp", bufs=4, space="PSUM"))
        ph1_p = ctx.enter_context(tc.tile_pool(name="ph1", bufs=2, space="PSUM"))
        ph2_p = ctx.enter_context(tc.tile_pool(name="ph2", bufs=1, space="PSUM"))
        po_p = ctx.enter_context(tc.tile_pool(name="po", bufs=1, space="PSUM"))

        # ---- constants / weights (loaded once, f32 staged -> bf16) ----
        identb = const.tile([128, 128], BF16)
        _make_identity_bf16(nc, identb[:])
        identf = const.tile([128, 128], F32)
        nc.gpsimd.memset(identf[:], 0.0)  # only used as ACT const-gen source

        # W1 lhsT tiles: w1_sb[p, ki, o] = W1[ki*128 + p, o], bf16
        w1_sb = const.tile([128, KI, H1], BF16)
        w1_re = w1_d.rearrange("(ki p) o -> p ki o", p=128)
        with tc.tile_pool(name="wtmp", bufs=1) as wtmp:
            for g in range(2):
                half = KI // 2
                tmp = wtmp.tile(
                    [128, half, H1], F32, tag="wtmp", bufs=1, name=f"wtmp{g}"
                )
                nc.sync.dma_start(tmp[:], w1_re[:, g * half : (g + 1) * half, :])
                nc.vector.tensor_copy(
                    w1_sb[:, g * half : (g + 1) * half, :], tmp[:]
                )

            # W2 lhsT tiles: w2_sb[p, ci, o] = W2[ci*128 + p, o], bf16
            w2_sb = const.tile([128, H1 // 128, H2], BF16)
            w2tmp = wtmp.tile([128, H1 // 128, H2], F32, name="w2tmp")
            nc.sync.dma_start(
                w2tmp[:], w2_d.rearrange("(ci p) o -> p ci o", p=128)
            )
            nc.vector.tensor_copy(w2_sb[:], w2tmp[:])

            # W3 extended [96, 32] bf16: zeros, then W3 block + b3 row
            w3x_sb = const.tile([K3, N_PAD], BF16)
            nc.scalar.activation(
                w3x_sb[:], identf[:K3, :N_PAD], IDENT, bias=0.0, scale=0.0
            )
            w3tmp = wtmp.tile([H2 + 1, N_CLS], F32, name="w3tmp")
            nc.sync.dma_start(w3tmp[:H2, :], w3_d)
            nc.sync.dma_start(
                w3tmp[H2 : H2 + 1, :], b3_d.rearrange("(a c) -> a c", a=1)
            )
            nc.vector.tensor_copy(w3x_sb[: H2 + 1, :N_CLS], w3tmp[:])

        # biases as per-partition f32 columns (ACT bias inputs)
        b1_sb = const.tile([128, H1 // 128], F32)
        nc.sync.dma_start(b1_sb[:], b1_d.rearrange("(oi p) -> p oi", p=128))
        b2_sb = const.tile([H2, 1], F32)
        nc.sync.dma_start(b2_sb[:], b2_d.rearrange("(c a) -> c a", a=1))

        # ---- main loop over row blocks ----
        for blk in range(N_BLK * repeat):
            r0 = (blk % N_BLK) * BLK

            # stream x naturally, convert f32 -> bf16 (DVE/ACT alternating)
            xb = []
            cvt_i = 0
            for dc in range(DC):
                row = []
                for rs in range(RSUB):
                    t = xnat_p.tile([128, DCW], F32, tag="xn", bufs=5)
                    nc.sync.dma_start(
                        t[:],
                        x_d[
                            r0 + rs * 128 : r0 + (rs + 1) * 128,
                            dc * DCW : (dc + 1) * DCW,
                        ],
                    )
                    tb = xbf_p.tile([128, DCW], BF16, tag="xb", bufs=10)
                    if cvt_i % 2 == 0:
                        nc.vector.tensor_copy(tb[:], t[:])
                    else:
                        nc.scalar.activation(tb[:], t[:], IDENT, bias=0.0)
                    cvt_i += 1
                    row.append(tb)
                xb.append(row)

            ph1 = []
            for oi in range(H1 // 128):
                pt = ph1_p.tile([128, BLK], F32, tag="ph1", bufs=2)
                ph1.append(pt)

            # k-chunks processed in pairs: one bf16 PSUM bank holds 2 chunks
            for kp in range(KI // 2):
                ptp = ptp_p.tile([128, 2, BLK], BF16, tag="ptp", bufs=4)
                for m in range(2):
                    ki = kp * 2 + m
                    dc, kl = divmod(ki, KI_PER_DC)
                    for rs in range(RSUB):
                        nc.tensor.transpose(
                            ptp[:, m, rs * 128 : (rs + 1) * 128],
                            xb[dc][rs][:, kl * 128 : (kl + 1) * 128],
                            identb[:],
                        )
                xt = xt_p.tile([128, 2, BLK], BF16, tag="xt", bufs=6)
                nc.vector.tensor_copy(xt[:], ptp[:])
                for m in range(2):
                    ki = kp * 2 + m
                    for oi in range(H1 // 128):
                        nc.tensor.matmul(
                            ph1[oi][:],
                            w1_sb[:, ki, oi * 128 : (oi + 1) * 128],
                            xt[:, m, :],
                            start=(ki == 0),
                            stop=(ki == KI - 1),
                        )

            # h1^T = relu(psum + b1): [256, 512] as two bf16 tiles
            h1t = []
            for oi in range(H1 // 128):
                ht = h_p.tile([128, BLK], BF16, tag="h1t", bufs=4)
                nc.scalar.activation(
                    ht[:], ph1[oi][:], RELU, bias=b1_sb[:, oi : oi + 1]
                )
                h1t.append(ht)

            # L2 -> h2^T [64, 512] (+ padding rows for the L3 lhsT)
            ph2 = ph2_p.tile([H2, BLK], F32, tag="ph2", bufs=1)
            for ci in range(H1 // 128):
                nc.tensor.matmul(
                    ph2[:],
                    w2_sb[:, ci, :],
                    h1t[ci][:],
                    start=(ci == 0),
                    stop=(ci == H1 // 128 - 1),
                )
            h2t = h_p.tile([K3, BLK], BF16, tag="h2t", bufs=2)
            nc.scalar.activation(h2t[:H2, :], ph2[:], RELU, bias=b2_sb[:])
            # rows 64:96 zeros, then row 64 = ones (b3 trick)
            nc.scalar.activation(
                h2t[H2:K3, :], ph2[: K3 - H2, :], IDENT, bias=0.0, scale=0.0
            )
            nc.scalar.activation(
                h2t[H2 : H2 + 1, :], ph2[0:1, :], IDENT, bias=1.0, scale=0.0
            )

            # L3: natural-orientation output [128 rows, 32] per sub-tile
            po = po_p.tile([128, RSUB * N_PAD], F32, tag="po", bufs=1)
            for rs in range(RSUB):
                nc.tensor.matmul(
                    po[:, rs * N_PAD : (rs + 1) * N_PAD],
                    h2t[:, rs * 128 : (rs + 1) * 128],
                    w3x_sb[:],
                    start=True,
                    stop=True,
                )
            ot = o_p.tile([128, RSUB * N_CLS], F32, tag="ot", bufs=2)
            nc.vector.tensor_copy(
                ot[:].rearrange("p (rs c) -> p rs c", c=N_CLS),
                po[:].rearrange("p (rs c) -> p rs c", c=N_PAD)[:, :, :N_CLS],
            )
            nc.sync.dma_start(
                out_d[r0 : r0 + BLK, :].rearrange("(rs p) c -> p rs c", p=128),
                ot[:].rearrange("p (rs c) -> p rs c", c=N_CLS),
            )

    nc.compile()
    return nc


_NC_CACHE = None


def kernel(**inputs) -> np.ndarray:
    global _NC_CACHE
    if _NC_CACHE is None:
        _NC_CACHE = build_program()
    nc = _NC_CACHE

    x = np.ascontiguousarray(inputs["x"], dtype=np.float32).reshape(ROWS_TOTAL, D_IN)
    common = {
        "W1": np.ascontiguousarray(inputs["W1"], dtype=np.float32),
        "b1": np.ascontiguousarray(inputs["b1"], dtype=np.float32),
        "W2": np.ascontiguousarray(inputs["W2"], dtype=np.float32),
        "b2": np.ascontiguousarray(inputs["b2"], dtype=np.float32),
        "W3": np.ascontiguousarray(inputs["W3"], dtype=np.float32),
        "b3": np.ascontiguousarray(inputs["b3"], dtype=np.float32),
    }
    in_maps = [
        {"x": x[i * ROWS : (i + 1) * ROWS], **common} for i in range(N_CORES)
    ]
    res = run_bass_kernel_spmd(nc, in_maps, list(range(N_CORES)))
    out = np.concatenate([res.results[i]["out"] for i in range(N_CORES)], axis=0)
    return out.reshape(16, 512, N_CLS).astype(np.float32)
